# revision 13
# baseline (speedup 1.0000x reference)
"""Single-head causal attention (B=4, S=4096, E=1024, D=64) on 8 TRN2 NeuronCores.

Sharding: 8 cores = 4 batches x 2 roles. Within a batch, query rows are dealt
to the two cores in interleaved 256-row blocks (role r owns global blocks
2i+r, i=0..7). With kv extents rounded up to 512, both roles see the exact
same causal geometry -> one uniform SPMD program. Causality inside the
diagonal 512-tile is enforced with per-core 0/1 mask inputs.

The host passes activations transposed (E-major) so projections need no
on-device transpose: qT/kT/vT = W.T @ x.T with E on partitions. Projections
run in float32r (full-rate fp32 matmul); attention runs in bf16 with f32 PSUM
accumulation. Softmax skips max-subtraction (|scores/8| < ~6 for this data)
and gets the denominator from a ones-column appended to V.
"""

import numpy as np
import ml_dtypes

import concourse.bass as bass
import concourse.tile as tile
from concourse import bacc, mybir
from concourse.bass_utils import run_bass_kernel_spmd
from concourse.masks import make_identity

B, S, E, QD = 4, 4096, 1024, 64
N_CORES = 8
QBLK = 256            # query rows per block
NBLK = 8              # blocks per core
SQ = QBLK * NBLK      # 2048 query rows per core
KV_TILE = 512
F32 = mybir.dt.float32
BF16 = mybir.dt.bfloat16
F32R = mybir.dt.float32r
ACTF = mybir.ActivationFunctionType


def build_nc(skv=S, emit_order="interleave"):
    """Build the uniform SPMD program. skv = kv rows each core projects."""
    nkv = skv // KV_TILE  # kv tiles this core projects (8 for V1 full-kv)
    nc = bacc.Bacc(trn_type="TRN2")

    xqT = nc.dram_tensor("xqT", [E, SQ], F32, kind="ExternalInput")
    xkT = nc.dram_tensor("xkT", [E, skv], F32, kind="ExternalInput")
    xvT = nc.dram_tensor("xvT", [E, skv], F32, kind="ExternalInput")
    wqT = nc.dram_tensor("wqT", [E, QD], F32, kind="ExternalInput")
    wkT = nc.dram_tensor("wkT", [E, QD], F32, kind="ExternalInput")
    wvT = nc.dram_tensor("wvT", [E, QD], F32, kind="ExternalInput")
    bq = nc.dram_tensor("bq", [QD, 1], F32, kind="ExternalInput")
    bk = nc.dram_tensor("bk", [QD, 1], F32, kind="ExternalInput")
    bv = nc.dram_tensor("bv", [QD, 1], F32, kind="ExternalInput")
    masks = nc.dram_tensor("masks", [128, 4, QBLK], BF16, kind="ExternalInput")
    out = nc.dram_tensor("out", [SQ, QD], F32, kind="ExternalOutput")

    with tile.TileContext(nc) as tc:
        with (
            tc.tile_pool(name="consts", bufs=1) as consts,
            tc.tile_pool(name="xin", bufs=6) as xin,
            tc.tile_pool(name="persist", bufs=1) as persist,
            tc.tile_pool(name="vtmp", bufs=2) as vtmp,
            tc.tile_pool(name="expp", bufs=40) as expp,
            tc.tile_pool(name="fin", bufs=2) as fin,
            tc.tile_pool(name="pproj", bufs=2, space="PSUM") as pproj,
            tc.tile_pool(name="pvt", bufs=2, space="PSUM") as pvt,
            tc.tile_pool(name="psc", bufs=2, space="PSUM") as psc,
            tc.tile_pool(name="po", bufs=2, space="PSUM") as po,
        ):
            # ---- constants ----
            w_sb = {}
            for nm, th in (("q", wqT), ("k", wkT), ("v", wvT)):
                wf = consts.tile([128, 8, QD], F32, name=f"wf_{nm}")
                nc.sync.dma_start(
                    out=wf, in_=th[:, :].rearrange("(e p) d -> p e d", p=128)
                )
                w = consts.tile([128, 8, QD], BF16, name=f"w_{nm}")
                nc.vector.tensor_copy(out=w, in_=wf)
                w_sb[nm] = w
            b_sb = {}
            for nm, th in (("q", bq), ("k", bk), ("v", bv)):
                t = consts.tile([QD, 1], F32, name=f"b_{nm}")
                nc.sync.dma_start(out=t, in_=th[:, :])
                b_sb[nm] = t
            mask_sb = consts.tile([128, 4, QBLK], BF16)
            nc.sync.dma_start(out=mask_sb, in_=masks[:, :, :])
            ident = consts.tile([128, 128], BF16)
            make_identity(nc, ident)

            # ---- persistent projected tensors ----
            qT_sb = persist.tile([QD, SQ], BF16)          # [64, 2048]
            kT_sb = persist.tile([QD, S], BF16)           # [64, 4096]
            v_sb = persist.tile([128, S // 128, QD + 1], BF16)  # [128, 32, 65]
            nc.vector.memset(v_sb[:, :, QD : QD + 1], 1.0)

            def project(dst_psum, w, xT, col0, s):
                """dst_psum[64, 512] = W.T @ xT[:, 512s:512s+512] (bf16)."""
                for g in range(2):
                    xt = xin.tile([128, 4, KV_TILE], F32, name="xt", tag="xin")
                    nc.sync.dma_start(
                        out=xt,
                        in_=xT[512 * g : 512 * (g + 1), col0 : col0 + KV_TILE]
                        .rearrange("(eq p) c -> p eq c", p=128),
                    )
                    xtb = xin.tile([128, 4, KV_TILE], BF16, name="xtb", tag="xinb")
                    nc.vector.tensor_copy(out=xtb, in_=xt)
                    for eq in range(4):
                        e = 4 * g + eq
                        nc.tensor.matmul(
                            dst_psum,
                            lhsT=w[:, e, :],
                            rhs=xtb[:, eq, :],
                            start=(e == 0),
                            stop=(e == 7),
                        )

            def project_q_tile(s):
                ps = pproj.tile([QD, KV_TILE], F32, tag="pproj")
                project(ps, w_sb["q"], xqT, 512 * s, s)
                nc.vector.tensor_scalar_add(
                    out=qT_sb[:, 512 * s : 512 * (s + 1)], in0=ps,
                    scalar1=b_sb["q"][:, :],
                )

            def project_kv_tile(t):
                # kT
                ps = pproj.tile([QD, KV_TILE], F32, tag="pproj")
                project(ps, w_sb["k"], xkT, 512 * t, t)
                nc.vector.tensor_scalar_add(
                    out=kT_sb[:, 512 * t : 512 * (t + 1)], in0=ps,
                    scalar1=b_sb["k"][:, :],
                )
                # v: project, bias, then transpose to [sk, 64] layout
                pv = pproj.tile([QD, KV_TILE], F32, tag="pproj")
                project(pv, w_sb["v"], xvT, 512 * t, t)
                vt = vtmp.tile([QD, KV_TILE], BF16, tag="vtmp")
                nc.vector.tensor_scalar_add(
                    out=vt, in0=pv, scalar1=b_sb["v"][:, :]
                )
                for u in range(4):
                    pt = pvt.tile([128, QD], BF16, tag="pvt")
                    nc.tensor.transpose(
                        pt, vt[:, 128 * u : 128 * (u + 1)], ident[:QD, :QD]
                    )
                    nc.vector.tensor_copy(
                        out=v_sb[:, 4 * t + u, 0:QD], in_=pt
                    )

            def attention_block(i):
                # h=0's accumulation group must fully complete before h=1's
                # starts: start=True clears has_written for the WHOLE bank, so
                # interleaved groups in one bank corrupt each other.
                o = po.tile([128, 2, QD + 1], F32, tag="po")
                rhs_q = qT_sb[:, QBLK * i : QBLK * (i + 1)]
                n_chunks = 4 * (i + 1)
                exs = []
                for a in range(n_chunks):
                    t = a // 4
                    m = a % 4
                    sc = psc.tile([128, QBLK], F32, tag="psc")
                    nc.tensor.matmul(
                        sc,
                        lhsT=kT_sb[:, 128 * a : 128 * (a + 1)],
                        rhs=rhs_q,
                        start=True,
                        stop=True,
                    )
                    ex = expp.tile([128, QBLK], BF16, tag="expp")
                    nc.scalar.activation(out=ex, in_=sc, func=ACTF.Exp, scale=0.125)
                    if t == i:
                        nc.vector.tensor_mul(ex, ex, mask_sb[:, m, :])
                    exs.append(ex)
                for h in range(2):
                    for a in range(n_chunks):
                        nc.tensor.matmul(
                            o[:, h, :],
                            lhsT=exs[a][:, 128 * h : 128 * (h + 1)],
                            rhs=v_sb[:, a, :],
                            start=(a == 0),
                            stop=(a == n_chunks - 1),
                        )
                for h in range(2):
                    rec = fin.tile([128, 1], F32, tag="rec")
                    nc.vector.reciprocal(rec, o[:, h, QD : QD + 1])
                    ot = fin.tile([128, QD], F32, tag="ot")
                    nc.vector.tensor_scalar_mul(ot, o[:, h, 0:QD], rec)
                    r0 = QBLK * i + 128 * h
                    nc.sync.dma_start(out=out[r0 : r0 + 128, :], in_=ot)

            # ---- emission ----
            if emit_order == "interleave":
                for s in range(4):
                    project_q_tile(s)
                for t in range(nkv):
                    project_kv_tile(t)
                    if t < NBLK:
                        attention_block(t)
                for i in range(nkv, NBLK):
                    attention_block(i)
            else:
                for s in range(4):
                    project_q_tile(s)
                for t in range(nkv):
                    project_kv_tile(t)
                for i in range(NBLK):
                    attention_block(i)

    nc.compile()
    return nc


def shard_inputs(query, key, value, Wq, bq, Wk, bk, Wv, bv):
    """Build per-core input maps (host-side sharding only: slice/transpose)."""
    query = np.asarray(query, dtype=np.float32)
    key = np.asarray(key, dtype=np.float32)
    value = np.asarray(value, dtype=np.float32)
    wqT = np.ascontiguousarray(np.asarray(Wq, np.float32).T)  # [E, QD]
    wkT = np.ascontiguousarray(np.asarray(Wk, np.float32).T)
    wvT = np.ascontiguousarray(np.asarray(Wv, np.float32).T)
    bq_ = np.asarray(bq, np.float32).reshape(QD, 1)
    bk_ = np.asarray(bk, np.float32).reshape(QD, 1)
    bv_ = np.asarray(bv, np.float32).reshape(QD, 1)

    # role-specific diagonal masks: valid iff 128*m + p <= 256*r + f
    mask_r = []
    p = np.arange(128)[:, None]
    f = np.arange(QBLK)[None, :]
    for r in range(2):
        ms = np.stack(
            [(128 * m + p <= 256 * r + f) for m in range(4)], axis=1
        ).astype(ml_dtypes.bfloat16)
        mask_r.append(np.ascontiguousarray(ms))

    in_maps = []
    for c in range(N_CORES):
        b, r = c // 2, c % 2
        rows = np.concatenate(
            [np.arange(QBLK * (2 * i + r), QBLK * (2 * i + r) + QBLK)
             for i in range(NBLK)]
        )
        xqT = np.ascontiguousarray(query[b][rows].T)        # [E, 2048]
        xkT = np.ascontiguousarray(key[b].T)                # [E, 4096]
        xvT = np.ascontiguousarray(value[b].T)
        in_maps.append({
            "xqT": xqT, "xkT": xkT, "xvT": xvT,
            "wqT": wqT, "wkT": wkT, "wvT": wvT,
            "bq": bq_, "bk": bk_, "bv": bv_,
            "masks": mask_r[r],
        })
    return in_maps


_NC_CACHE = {}


def kernel(query, key, value, Wq, bq, Wk, bk, Wv, bv):
    if "nc" not in _NC_CACHE:
        _NC_CACHE["nc"] = build_nc(skv=S)
    nc = _NC_CACHE["nc"]
    in_maps = shard_inputs(query, key, value, Wq, bq, Wk, bk, Wv, bv)
    res = run_bass_kernel_spmd(nc, in_maps, core_ids=list(range(N_CORES)))
    out = np.empty((B, S, QD), np.float32)
    for c in range(N_CORES):
        b, r = c // 2, c % 2
        o = res.results[c]["out"]  # [2048, 64] local block order
        for i in range(NBLK):
            g0 = QBLK * (2 * i + r)
            out[b, g0 : g0 + QBLK] = o[QBLK * i : QBLK * (i + 1)]
    return out


# revision 15
# speedup vs baseline: 1.1084x; 1.1084x over previous
"""Single-head causal attention (B=4, S=4096, E=1024, D=64) on 8 TRN2 NeuronCores.

Sharding: 8 cores = 4 batches x 2 roles. Within a batch, query rows are dealt
to the two cores in interleaved 256-row blocks (role r owns global blocks
2i+r, i=0..7). With kv extents rounded up to 512, both roles see the exact
same causal geometry -> one uniform SPMD program. Causality inside the
diagonal tiles is enforced with per-core 0/1 mask inputs.

The host passes activations transposed (E-major) so projections need no
on-device transpose: qT/kT/vT = W.T @ x.T with E on partitions. Projections
run in float32r (full-rate fp32 matmul, no cast needed); attention runs in
bf16 with f32 PSUM accumulation. Softmax skips max-subtraction (|scores/8| <
~6 for this data) and gets the denominator from a ones-column appended to V.
Scores are computed per q 512-tile (two 256-blocks at once) to amortize
per-instruction overhead on PE and the exp on ACT.
"""

import numpy as np
import ml_dtypes

import concourse.bass as bass
import concourse.tile as tile
from concourse import bacc, mybir
from concourse.bass_utils import run_bass_kernel_spmd
from concourse.masks import make_identity

B, S, E, QD = 4, 4096, 1024, 64
N_CORES = 8
QBLK = 256            # query rows per block
NBLK = 8              # blocks per core
SQ = QBLK * NBLK      # 2048 query rows per core
KV_TILE = 512
F32 = mybir.dt.float32
BF16 = mybir.dt.bfloat16
F32R = mybir.dt.float32r
ACTF = mybir.ActivationFunctionType


def build_nc(skv=S, use_cc=False):
    """Build the uniform SPMD program. skv = kv rows each core projects."""
    nkv = skv // KV_TILE
    nc = bacc.Bacc(trn_type="TRN2", num_devices=N_CORES)

    xqT = nc.dram_tensor("xqT", [E, SQ], F32R, kind="ExternalInput")
    xkT = nc.dram_tensor("xkT", [E, skv], F32R, kind="ExternalInput")
    xvT = nc.dram_tensor("xvT", [E, skv], F32R, kind="ExternalInput")
    wqT = nc.dram_tensor("wqT", [E, QD], F32R, kind="ExternalInput")
    wkT = nc.dram_tensor("wkT", [E, QD], F32R, kind="ExternalInput")
    wvT = nc.dram_tensor("wvT", [E, QD], F32R, kind="ExternalInput")
    bq = nc.dram_tensor("bq", [QD, 1], F32, kind="ExternalInput")
    bk = nc.dram_tensor("bk", [QD, 1], F32, kind="ExternalInput")
    bv = nc.dram_tensor("bv", [QD, 1], F32, kind="ExternalInput")
    masks = nc.dram_tensor("masks", [128, 8, KV_TILE], BF16, kind="ExternalInput")
    out = nc.dram_tensor("out", [SQ, QD], F32, kind="ExternalOutput")

    with tile.TileContext(nc) as tc:
        with (
            tc.tile_pool(name="consts", bufs=1) as consts,
            tc.tile_pool(name="xin", bufs=6) as xin,
            tc.tile_pool(name="persist", bufs=1) as persist,
            tc.tile_pool(name="vtmp", bufs=2) as vtmp,
            tc.tile_pool(name="expp", bufs=34) as expp,
            tc.tile_pool(name="fin", bufs=4) as fin,
            tc.tile_pool(name="pproj", bufs=2, space="PSUM") as pproj,
            tc.tile_pool(name="pvt", bufs=2, space="PSUM") as pvt,
            tc.tile_pool(name="psc", bufs=2, space="PSUM") as psc,
            tc.tile_pool(name="po", bufs=2, space="PSUM") as po,
            tc.tile_pool(name="dram", bufs=1, space="DRAM") as dram,
        ):
            # ---- constants ----
            w_sb = {}
            for nm, th in (("q", wqT), ("k", wkT), ("v", wvT)):
                w = consts.tile([128, 8, QD], F32R, name=f"w_{nm}")
                nc.sync.dma_start(
                    out=w, in_=th[:, :].rearrange("(e p) d -> p e d", p=128)
                )
                w_sb[nm] = w
            b_sb = {}
            for nm, th in (("q", bq), ("k", bk), ("v", bv)):
                t = consts.tile([QD, 1], F32, name=f"b_{nm}")
                nc.sync.dma_start(out=t, in_=th[:, :])
                b_sb[nm] = t
            mask_sb = consts.tile([128, 8, KV_TILE], BF16)
            nc.sync.dma_start(out=mask_sb, in_=masks[:, :, :])
            ident = consts.tile([128, 128], BF16)
            make_identity(nc, ident)

            # ---- persistent projected tensors ----
            qT_sb = persist.tile([QD, SQ], BF16)          # [64, 2048]
            kT_sb = persist.tile([QD, S], BF16)           # [64, 4096]
            v_sb = persist.tile([128, S // 128, QD + 1], BF16)  # [128, 32, 65]
            nc.vector.memset(v_sb[:, :, QD : QD + 1], 1.0)

            if use_cc:
                cc_k = KV_TILE * QD                        # 32768 bf16 elems
                cc_v = 4 * 128 * (QD + 1)                  # 33280
                cc_len = cc_k + cc_v
                cc_ins = [dram.tile([1, cc_len], BF16, name=f"ccin{p}")
                          for p in range(4)]
                cc_outs = [dram.tile([2, cc_len], BF16, name=f"ccout{p}")
                           for p in range(4)]

            def project(dst_psum, w, xT, col0):
                """dst_psum[64, 512] = W.T @ xT[:, col0:col0+512] (f32r)."""
                for g in range(2):
                    xt = xin.tile([128, 4, KV_TILE], F32R, name="xt", tag="xin")
                    nc.sync.dma_start(
                        out=xt,
                        in_=xT[512 * g : 512 * (g + 1), col0 : col0 + KV_TILE]
                        .rearrange("(eq p) c -> p eq c", p=128),
                    )
                    for eq in range(4):
                        e = 4 * g + eq
                        nc.tensor.matmul(
                            dst_psum,
                            lhsT=w[:, e, :],
                            rhs=xt[:, eq, :],
                            start=(e == 0),
                            stop=(e == 7),
                        )

            def project_q_tile(s):
                ps = pproj.tile([QD, KV_TILE], F32, tag="pproj")
                project(ps, w_sb["q"], xqT, 512 * s)
                nc.vector.tensor_scalar_add(
                    out=qT_sb[:, 512 * s : 512 * (s + 1)], in0=ps,
                    scalar1=b_sb["q"][:, :],
                )

            def project_kv_tile(t, kT_dst, v_dst):
                """Project local kv tile t into kT_dst [64,512] and
                v_dst [128, 4, 65] (both bf16 SBUF APs)."""
                ps = pproj.tile([QD, KV_TILE], F32, tag="pproj")
                project(ps, w_sb["k"], xkT, 512 * t)
                nc.vector.tensor_scalar_add(
                    out=kT_dst, in0=ps, scalar1=b_sb["k"][:, :]
                )
                pv = pproj.tile([QD, KV_TILE], F32, tag="pproj")
                project(pv, w_sb["v"], xvT, 512 * t)
                vt = vtmp.tile([QD, KV_TILE], BF16, tag="vtmp")
                nc.vector.tensor_scalar_add(out=vt, in0=pv, scalar1=b_sb["v"][:, :])
                for u in range(4):
                    pt = pvt.tile([128, QD], BF16, tag="pvt")
                    nc.tensor.transpose(
                        pt, vt[:, 128 * u : 128 * (u + 1)], ident[:QD, :QD]
                    )
                    nc.vector.tensor_copy(out=v_dst[:, u, 0:QD], in_=pt)

            def project_kv_local(t):
                project_kv_tile(
                    t,
                    kT_sb[:, 512 * t : 512 * (t + 1)],
                    v_sb[:, 4 * t : 4 * t + 4, :],
                )

            def project_kv_cc(p):
                """Project own piece p, all-gather pair-wide, scatter into
                kT_sb / v_sb (global tiles p and 4+p... global g = 4*src+p)."""
                kp = vtmp.tile([QD, KV_TILE], BF16, tag="kpiece")
                vp = vtmp.tile([128, 4, QD + 1], BF16, tag="vpiece")
                nc.vector.memset(vp[:, :, QD : QD + 1], 1.0)
                project_kv_tile(p, kp[:, :], vp)
                cin, cout = cc_ins[p], cc_outs[p]
                k_ap = cin[0, 0:cc_k].rearrange("(d c) -> d c", d=QD)
                v_ap = cin[0, cc_k:cc_len].rearrange("(p a c) -> p a c", p=128, a=4)
                nc.sync.dma_start(out=k_ap, in_=kp[:, :])
                nc.sync.dma_start(out=v_ap, in_=vp[:, :, :])
                nc.gpsimd.collective_compute(
                    "AllGather",
                    mybir.AluOpType.bypass,
                    replica_groups=[[0, 1], [2, 3], [4, 5], [6, 7]],
                    ins=[cin[:, :]],
                    outs=[cout[:, :]],
                )
                for src in range(2):
                    g = 4 * src + p
                    ko = cout[src, 0:cc_k].rearrange("(d c) -> d c", d=QD)
                    vo = cout[src, cc_k:cc_len].rearrange(
                        "(p a c) -> p a c", p=128, a=4
                    )
                    nc.sync.dma_start(
                        out=kT_sb[:, 512 * g : 512 * (g + 1)], in_=ko
                    )
                    nc.sync.dma_start(out=v_sb[:, 4 * g : 4 * g + 4, :], in_=vo)

            def attention_pair(s):
                """Blocks 2s and 2s+1 (local) against kv chunks [0, 8s+8)."""
                o = po.tile([128, 4, QD + 1], F32, tag="po")
                rhs_q = qT_sb[:, 512 * s : 512 * (s + 1)]
                n_chunks = 8 * s + 8
                exs = []
                for a in range(n_chunks):
                    sc = psc.tile([128, KV_TILE], F32, tag="psc")
                    nc.tensor.matmul(
                        sc,
                        lhsT=kT_sb[:, 128 * a : 128 * (a + 1)],
                        rhs=rhs_q,
                        start=True,
                        stop=True,
                    )
                    ex = expp.tile([128, KV_TILE], BF16, tag="expp")
                    nc.scalar.activation(out=ex, in_=sc, func=ACTF.Exp, scale=0.125)
                    j = a - 8 * s
                    if j >= 0:
                        nc.vector.tensor_mul(ex, ex, mask_sb[:, j, :])
                    exs.append(ex)
                # accumulation groups must be sequential per PSUM bank:
                # start=True clears has_written for the whole bank.
                for bp in range(2):
                    nch_b = 4 * (2 * s + bp + 1)
                    for h in range(2):
                        g = 2 * bp + h
                        col = 256 * bp + 128 * h
                        for a in range(nch_b):
                            nc.tensor.matmul(
                                o[:, g, :],
                                lhsT=exs[a][:, col : col + 128],
                                rhs=v_sb[:, a, :],
                                start=(a == 0),
                                stop=(a == nch_b - 1),
                            )
                for bp in range(2):
                    i = 2 * s + bp
                    for h in range(2):
                        g = 2 * bp + h
                        rec = fin.tile([128, 1], F32, tag="rec")
                        nc.vector.reciprocal(rec, o[:, g, QD : QD + 1])
                        ot = fin.tile([128, QD], F32, tag="ot")
                        nc.vector.tensor_scalar_mul(ot, o[:, g, 0:QD], rec)
                        r0 = QBLK * i + 128 * h
                        nc.sync.dma_start(out=out[r0 : r0 + 128, :], in_=ot)

            # ---- emission ----
            if use_cc:
                for p in range(4):
                    project_kv_cc(p)
                    project_q_tile(p)
                for s in range(4):
                    attention_pair(s)
            else:
                for s in range(4):
                    project_q_tile(s)
                    project_kv_local(2 * s)
                    project_kv_local(2 * s + 1)
                    attention_pair(s)

    nc.compile()
    return nc


def shard_inputs(query, key, value, Wq, bq, Wk, bk, Wv, bv, use_cc=False):
    """Build per-core input maps (host-side sharding only: slice/transpose)."""
    query = np.asarray(query, dtype=np.float32)
    key = np.asarray(key, dtype=np.float32)
    value = np.asarray(value, dtype=np.float32)
    wqT = np.ascontiguousarray(np.asarray(Wq, np.float32).T)  # [E, QD]
    wkT = np.ascontiguousarray(np.asarray(Wk, np.float32).T)
    wvT = np.ascontiguousarray(np.asarray(Wv, np.float32).T)
    bq_ = np.asarray(bq, np.float32).reshape(QD, 1)
    bk_ = np.asarray(bk, np.float32).reshape(QD, 1)
    bv_ = np.asarray(bv, np.float32).reshape(QD, 1)

    # role-specific diagonal masks [128, 8, 512]:
    # col f covers block-pair: q_off = 512*(f//256) + 256*r + f%256
    # pattern j valid iff 128*j + p <= q_off
    p = np.arange(128)[:, None]
    f = np.arange(KV_TILE)[None, :]
    mask_r = []
    for r in range(2):
        q_off = 512 * (f // 256) + 256 * r + (f % 256)
        ms = np.stack(
            [(128 * j + p <= q_off) for j in range(8)], axis=1
        ).astype(ml_dtypes.bfloat16)
        mask_r.append(np.ascontiguousarray(ms))

    in_maps = []
    for c in range(N_CORES):
        b, r = c // 2, c % 2
        rows = np.concatenate(
            [np.arange(QBLK * (2 * i + r), QBLK * (2 * i + r) + QBLK)
             for i in range(NBLK)]
        )
        xqT = np.ascontiguousarray(query[b][rows].T)        # [E, 2048]
        if use_cc:
            kv_rows = slice(2048 * r, 2048 * (r + 1))
            xkT = np.ascontiguousarray(key[b, kv_rows].T)   # [E, 2048]
            xvT = np.ascontiguousarray(value[b, kv_rows].T)
        else:
            xkT = np.ascontiguousarray(key[b].T)            # [E, 4096]
            xvT = np.ascontiguousarray(value[b].T)
        in_maps.append({
            "xqT": xqT, "xkT": xkT, "xvT": xvT,
            "wqT": wqT, "wkT": wkT, "wvT": wvT,
            "bq": bq_, "bk": bk_, "bv": bv_,
            "masks": mask_r[r],
        })
    return in_maps


USE_CC = False
_NC_CACHE = {}


def kernel(query, key, value, Wq, bq, Wk, bk, Wv, bv):
    if "nc" not in _NC_CACHE:
        _NC_CACHE["nc"] = build_nc(
            skv=(S // 2 if USE_CC else S), use_cc=USE_CC
        )
    nc = _NC_CACHE["nc"]
    in_maps = shard_inputs(
        query, key, value, Wq, bq, Wk, bk, Wv, bv, use_cc=USE_CC
    )
    res = run_bass_kernel_spmd(nc, in_maps, core_ids=list(range(N_CORES)))
    out = np.empty((B, S, QD), np.float32)
    for c in range(N_CORES):
        b, r = c // 2, c % 2
        o = res.results[c]["out"]  # [2048, 64] local block order
        for i in range(NBLK):
            g0 = QBLK * (2 * i + r)
            out[b, g0 : g0 + QBLK] = o[QBLK * i : QBLK * (i + 1)]
    return out
